# revision 21
# baseline (speedup 1.0000x reference)
"""Trainium2 Bass kernel for nn_CP_Based (CP-decomposition interaction layer).

Math (full problem):
    t[b,f,r,u] = sum_d X[b,f,d] * K[d,r,f,u]      (B=1024, F=64, D=4, R=32, U=128)
    had[b,r,u] = prod_f t[b,f,r,u]
    out[b,u]   = sum_r had[b,r,u]

Strategy (v6):
  * Shard batch x units across 8 cores as (2 batch halves) x (4 unit
    quarters): per core B_loc=512 (4 partition tiles of 128) and RU_loc =
    32r x 32u = 1024 columns (u-major, r contiguous for the final reduce).
  * Host-side feature grouping: 16 triples (K=64, row-group pairs sharing a
    kt slot) + 4 quads (K=256 as 2 PSUM-accumulated K=128 passes) = 20
    factor tiles [128,1024] per batch tile.  Matmul inputs fp16.
  * PSUM ring: narrow [128,1024] tiles, bufs=4 (8 banks).  ALL consumers are
    narrow single-tile ops so every bank releases independently at ~PE pace
    (wide 2-slot rings serialize mm->consume and pace the whole btile).
  * Consumers: 7 tiles (incl. ALL 4 quads - numerically the touchiest) fold
    into a narrow fp32 fused chain G on DVE (one PSUM operand per op, no
    16-bit quantization).  13 tiles drain via Act ACTIVATE into bf16, packed
    pairwise into [128,2048] wide buffers so the product tree runs as wide
    bf16 muls in the DVE 2x tensor_tensor mode (~1.22us/2048 cols).  bf16,
    not fp16: DVE's fast 16-bit uop programs are bf16-only.
  * Tree: DVE chains 3 wide bufs, Pool chains the other 3 (2 wide muls),
    DVE combines + folds + handles the leftover narrow tile; the final
    merge with G runs on Pool and the strided r-reduce on DVE.  Cross-btile
    tail ops ride DVE's idle ramp of the next btile; finalization is
    deferred one btile (emitted mid-btile so queues never head-of-line
    block).
  * Input DMA split across two DGE rings (sync + gpsimd) in consumption
    order so the first matmul starts ~2.5us in.
"""

import numpy as np

B, F, D, R, U = 1024, 64, 4, 32, 128
NCORES = 8
BSH, USH = 2, 4                 # batch shards x unit shards
BLOC = B // BSH                 # 512 batch rows per core
NBT = BLOC // 128               # 4 batch tiles of 128
ULOC = U // USH                 # 32 units per core
RUL = R * ULOC                  # 1024 columns (u-major: col = u*32 + r)
NQ = 4                          # quads (features 0..15)
NT = 16                         # triples (features 16..63)
NTILE = NT + NQ                 # 20 factor tiles per batch tile
NPAIR = NT // 2                 # triple pairs (kt slots 0..7)
NSLOT = NPAIR + 2 * NQ          # kt slots: 8 triple-pairs + 2 per quad

FUSED = (2, 5, 8, 16, 17, 18, 19)   # DVE fp32 fused chain (all quads anchored)
ACTS = tuple(i for i in range(NTILE) if i not in FUSED)  # 13 Act drains
# pack consecutive Act tiles into halves of wide bf16 buffers
APAIR = tuple((ACTS[2 * k], ACTS[2 * k + 1]) for k in range(len(ACTS) // 2))
ALONE = ACTS[-1]                # leftover narrow tile (15)
# production order: quads interleaved mid-btile so their serial DVE G-ops
# land in DVE's slack window; btile tail is all fast-release Act drains
PORDER = (0, 1, 2, 3, 16, 4, 5, 17, 6, 7, 18, 8, 9, 19, 10, 11, 12, 13, 14, 15)

_cached = {}


def _build_nc():
    import concourse.bass as bass
    import concourse.mybir as mybir
    import concourse.tile as tile
    from concourse import bacc

    fp32 = mybir.dt.float32
    fp16 = mybir.dt.float16
    bf16 = mybir.dt.bfloat16
    nc = bacc.Bacc("TRN2", target_bir_lowering=False, debug=False)

    xt_d = nc.dram_tensor("xt", [NBT, 128, NSLOT * 128], fp16, kind="ExternalInput").ap()
    kt_d = nc.dram_tensor("kt", [NSLOT, 128, RUL], fp16, kind="ExternalInput").ap()
    out_d = nc.dram_tensor("out", [BLOC, ULOC], fp32, kind="ExternalOutput").ap()

    W2 = 2 * RUL

    # map act tile -> (pair index, half) or None for the leftover
    half_of = {}
    for k, (i, j) in enumerate(APAIR):
        half_of[i] = (k, 0)
        half_of[j] = (k, 1)

    with tile.TileContext(nc) as tc:
        with (
            tc.tile_pool(name="kt", bufs=1) as ktpool,
            tc.tile_pool(name="xt", bufs=1) as xtpool,
            tc.tile_pool(name="fb", bufs=12) as fbpool,
            tc.tile_pool(name="fn", bufs=2) as fnpool,
            tc.tile_pool(name="gg", bufs=2) as ggpool,
            tc.tile_pool(name="tt", bufs=2) as ttpool,
            tc.tile_pool(name="uu", bufs=2) as uupool,
            tc.tile_pool(name="vv", bufs=2) as vvpool,
            tc.tile_pool(name="oo", bufs=2) as oopool,
            tc.tile_pool(name="out", bufs=2) as outpool,
            tc.tile_pool(name="ps", bufs=4, space="PSUM") as pspool,
        ):
            xts = [
                xtpool.tile([128, NSLOT * 128], fp16, tag=f"xt{t}", name=f"xt{t}")
                for t in range(NBT)
            ]
            kts = [
                ktpool.tile([128, RUL], fp16, tag=f"kt{s}", name=f"kt{s}")
                for s in range(NSLOT)
            ]
            # three DGE rings: sync + scalar (HW) and gpsimd (SW).  First
            # matmul needs xt0 cols 0:128 + kt0; split xt0 into consumption-
            # ordered column chunks and fan the urgent transfers out.
            # kt slots ordered by first use under PORDER; three DGE rings
            nc.scalar.dma_start(kts[0][:], kt_d[0])
            nc.sync.dma_start(xts[0][:], xt_d[0])
            for s in (1, 9, 2, 11, 3, 13, 4, 15):
                nc.gpsimd.dma_start(kts[s][:], kt_d[s])
            for s in (8, 10, 12, 14, 5, 6, 7):
                nc.sync.dma_start(kts[s][:], kt_d[s])
            for t in range(1, NBT):
                nc.sync.dma_start(xts[t][:], xt_d[t])

            pending = []

            def xsl(s):
                return slice(s * 128, (s + 1) * 128)

            for t in range(NBT):
                xt = xts[t]

                Gb = [
                    ggpool.tile([128, RUL], fp32, tag=f"G{i}", name=f"G{i}")
                    for i in range(2)
                ]
                Tb = [
                    ttpool.tile([128, W2], bf16, tag=f"T{i}", name=f"T{i}")
                    for i in range(2)
                ]
                Ub = [
                    uupool.tile([128, W2], bf16, tag=f"U{i}", name=f"U{i}")
                    for i in range(2)
                ]
                Vw = vvpool.tile([128, W2], bf16, tag="V", name="V")
                Sn = vvpool.tile([128, RUL], bf16, tag="Sn", name="Sn")
                Sn2 = vvpool.tile([128, RUL], bf16, tag="Sn2", name="Sn2")
                fbs = [
                    fbpool.tile([128, W2], bf16, tag="fb", name="fb")
                    for _ in range(len(APAIR))
                ]
                fnn = fnpool.tile([128, RUL], bf16, tag="fn", name="fn")

                ng = 0
                for i in PORDER:
                    ps = pspool.tile([128, RUL], fp32, tag="ps", name="ps")
                    if i < NT:  # triple half: pair p = i//2, sub s = i%2
                        p, s = divmod(i, 2)
                        rows = slice(64 * s, 64 * s + 64)
                        for h in range(2):
                            cs = slice(512 * h, 512 * h + 512)
                            nc.tensor.matmul(
                                ps[:, cs],
                                xt[rows, xsl(p)],
                                kts[p][rows, cs],
                                start=True,
                                stop=True,
                                tile_position=(64 * s, 0),
                            )
                    else:  # quad: 2 psum-accumulated K=128 passes
                        q = i - NT
                        for h in range(2):
                            slot = NPAIR + 2 * q + h
                            for c in range(2):
                                cs = slice(512 * c, 512 * c + 512)
                                nc.tensor.matmul(
                                    ps[:, cs],
                                    xt[:, xsl(slot)],
                                    kts[slot][:, cs],
                                    start=(h == 0),
                                    stop=(h == 1),
                                )

                    if i in FUSED:
                        if ng == 0:
                            nc.vector.tensor_copy(Gb[0][:], ps[:])
                        else:
                            nc.vector.tensor_mul(
                                Gb[ng % 2][:], Gb[(ng - 1) % 2][:], ps[:]
                            )
                        ng += 1
                    elif i == ALONE:
                        nc.scalar.copy(fnn[:], ps[:])
                        # on Pool: a DVE op here hits a pathological slow
                        # path (~3.9us) and head-of-line blocks the next
                        # btile's G-chain; Pool is idle at btile end anyway
                        nc.gpsimd.tensor_mul(Sn2[:], Sn[:], fnn[:])
                    else:
                        k, h = half_of[i]
                        nc.scalar.copy(fbs[k][:, h * RUL : (h + 1) * RUL], ps[:])
                        # Pool (slow, 4.1us/wide) gets the EARLY buffers so it
                        # finishes within the btile; DVE (fast) gets the LATE
                        # ones so the cross-btile tail is short.
                        if i == APAIR[1][1]:       # B0,B1 drained
                            nc.gpsimd.tensor_mul(Ub[0][:], fbs[0][:], fbs[1][:])
                        elif i == APAIR[2][1]:     # B2 drained
                            nc.gpsimd.tensor_mul(Ub[1][:], Ub[0][:], fbs[2][:])
                        elif i == APAIR[4][1]:     # B3,B4 drained
                            nc.vector.tensor_mul(Tb[0][:], fbs[3][:], fbs[4][:])
                        elif i == APAIR[5][1]:     # B5 drained
                            nc.vector.tensor_mul(Tb[1][:], Tb[0][:], fbs[5][:])

                    if i == 2 and pending:
                        pending.pop(0)()

                    # tail right after T1 (Sn2 follows at the ALONE drain,
                    # which is the last position)
                    if i == APAIR[5][1]:
                        nc.vector.tensor_mul(Vw[:], Tb[1][:], Ub[1][:])
                        nc.vector.tensor_mul(Sn[:], Vw[:, 0:RUL], Vw[:, RUL:W2])

                def finalize(t=t, G=Gb[(len(FUSED) - 1) % 2], Sn2=Sn2):
                    O = oopool.tile([128, RUL], fp32, tag="O", name="O")
                    osum = outpool.tile([128, ULOC], fp32, tag="osum", name="osum")
                    nc.gpsimd.tensor_mul(O[:], G[:], Sn2[:])
                    nc.vector.tensor_reduce(
                        osum[:],
                        O[:].rearrange("p (u r) -> p u r", r=R),
                        axis=mybir.AxisListType.X,
                        op=mybir.AluOpType.add,
                    )
                    nc.sync.dma_start(out_d[t * 128 : (t + 1) * 128, :], osum[:])

                pending.append(finalize)

            for fin in pending:
                fin()

    nc.compile()
    return nc


def _host_prep(X, K):
    """Repack inputs into per-core fp16 stationary/moving operands.

    Quad q covers features 4q..4q+3 as two K=128 PSUM-accumulated passes
    (row = ((d0*4+d1)*4+d2)*2 + l, l indexing half of the 4th feature's
    d range).  Triples cover features 48+3j..50+3j (row = d0*16+d1*4+d2),
    two per kt slot (rows 0:64 / 64:128) for row-tiled matmul pairs;
    feature 63 rides in the third pair's B half (rows 64:68).  Columns are
    u-major (col = u*32 + r).
    """
    f16 = np.float16
    FT = 4 * NQ                      # first triple feature
    kt_cores, xt_cores = [], []
    for bi in range(BSH):
        Xc = X[bi * BLOC : (bi + 1) * BLOC]                    # [512, 64, 4]
        for uj in range(USH):
            Ku = K[:, :, :, uj * ULOC : (uj + 1) * ULOC]       # [4,32,64,32]
            Kf = np.ascontiguousarray(
                Ku.transpose(2, 0, 3, 1).reshape(F, D, RUL)
            )                                                   # [f, d, col]
            kt = np.zeros((NSLOT, 128, RUL), dtype=f16)
            xt = np.zeros((NBT, 128, NSLOT * 128), dtype=f16)

            def put_x(slot, rows, arr):  # arr [BLOC, nrows]
                for t in range(NBT):
                    xt[t, rows, slot * 128 : (slot + 1) * 128] = arr[
                        t * 128 : (t + 1) * 128
                    ].T

            # triple pairs in slots 0..NPAIR-1
            for p in range(NPAIR):
                for s in range(2):
                    j = 2 * p + s
                    rows = slice(64 * s, 64 * s + 64)
                    f0 = FT + 3 * j
                    K3 = (
                        Kf[f0][:, None, None, :]
                        * Kf[f0 + 1][None, :, None, :]
                        * Kf[f0 + 2][None, None, :, :]
                    ).reshape(64, RUL)
                    X3 = (
                        Xc[:, f0, :, None, None]
                        * Xc[:, f0 + 1, None, :, None]
                        * Xc[:, f0 + 2, None, None, :]
                    ).reshape(BLOC, 64)
                    kt[p, rows] = K3
                    put_x(p, rows, X3)
            # quads in slots NPAIR + 2q + h
            for q in range(NQ):
                f0 = 4 * q
                K012 = (
                    Kf[f0][:, None, None, :]
                    * Kf[f0 + 1][None, :, None, :]
                    * Kf[f0 + 2][None, None, :, :]
                ).reshape(64, RUL)
                X012 = (
                    Xc[:, f0, :, None, None]
                    * Xc[:, f0 + 1, None, :, None]
                    * Xc[:, f0 + 2, None, None, :]
                ).reshape(BLOC, 64)
                for h in range(2):
                    slot = NPAIR + 2 * q + h
                    kt[slot] = (
                        K012[:, None, :] * Kf[f0 + 3][2 * h : 2 * h + 2][None, :, :]
                    ).reshape(128, RUL)
                    X4h = (
                        X012[:, :, None]
                        * Xc[:, f0 + 3, 2 * h : 2 * h + 2][:, None, :]
                    ).reshape(BLOC, 128)
                    put_x(slot, slice(0, 128), X4h)
            kt_cores.append(np.ascontiguousarray(kt))
            xt_cores.append(np.ascontiguousarray(xt))
    return [{"xt": xt_cores[c], "kt": kt_cores[c]} for c in range(NCORES)]


def kernel(**inputs):
    from concourse.bass_utils import run_bass_kernel_spmd

    X = np.asarray(inputs["X"], dtype=np.float32)
    K = np.asarray(inputs["kernel"], dtype=np.float32)
    assert X.shape == (B, F, D) and K.shape == (D, R, F, U)

    if "nc" not in _cached:
        _cached["nc"] = _build_nc()
    nc = _cached["nc"]

    in_maps = _host_prep(X, K)
    res = run_bass_kernel_spmd(nc, in_maps, core_ids=list(range(NCORES)))
    out = np.zeros((B, U), dtype=np.float32)
    for c in range(NCORES):
        bi, uj = divmod(c, USH)
        out[bi * BLOC : (bi + 1) * BLOC, uj * ULOC : (uj + 1) * ULOC] = res.results[
            c
        ]["out"]
    return out


# revision 25
# speedup vs baseline: 1.0133x; 1.0133x over previous
"""Trainium2 Bass kernel for nn_CP_Based (CP-decomposition interaction layer).

Math (full problem):
    t[b,f,r,u] = sum_d X[b,f,d] * K[d,r,f,u]      (B=1024, F=64, D=4, R=32, U=128)
    had[b,r,u] = prod_f t[b,f,r,u]
    out[b,u]   = sum_r had[b,r,u]

Strategy (v6):
  * Shard batch x units across 8 cores as (2 batch halves) x (4 unit
    quarters): per core B_loc=512 (4 partition tiles of 128) and RU_loc =
    32r x 32u = 1024 columns (u-major, r contiguous for the final reduce).
  * Host-side feature grouping: 16 triples (K=64, row-group pairs sharing a
    kt slot) + 4 quads (K=256 as 2 PSUM-accumulated K=128 passes) = 20
    factor tiles [128,1024] per batch tile.  Matmul inputs fp16.
  * PSUM ring: narrow [128,1024] tiles, bufs=4 (8 banks).  ALL consumers are
    narrow single-tile ops so every bank releases independently at ~PE pace
    (wide 2-slot rings serialize mm->consume and pace the whole btile).
  * Consumers: 7 tiles (incl. ALL 4 quads - numerically the touchiest) fold
    into a narrow fp32 fused chain G on DVE (one PSUM operand per op, no
    16-bit quantization).  13 tiles drain via Act ACTIVATE into bf16, packed
    pairwise into [128,2048] wide buffers so the product tree runs as wide
    bf16 muls in the DVE 2x tensor_tensor mode (~1.22us/2048 cols).  bf16,
    not fp16: DVE's fast 16-bit uop programs are bf16-only.
  * Tree: DVE chains 3 wide bufs, Pool chains the other 3 (2 wide muls),
    DVE combines + folds + handles the leftover narrow tile; the final
    merge with G runs on Pool and the strided r-reduce on DVE.  Cross-btile
    tail ops ride DVE's idle ramp of the next btile; finalization is
    deferred one btile (emitted mid-btile so queues never head-of-line
    block).
  * Input DMA split across two DGE rings (sync + gpsimd) in consumption
    order so the first matmul starts ~2.5us in.
"""

import numpy as np

B, F, D, R, U = 1024, 64, 4, 32, 128
NCORES = 8
BSH, USH = 2, 4                 # batch shards x unit shards
BLOC = B // BSH                 # 512 batch rows per core
NBT = BLOC // 128               # 4 batch tiles of 128
ULOC = U // USH                 # 32 units per core
RUL = R * ULOC                  # 1024 columns (u-major: col = u*32 + r)
NQ = 4                          # quads (features 0..15)
NT = 16                         # triples (features 16..63)
NTILE = NT + NQ                 # 20 factor tiles per batch tile
NPAIR = NT // 2                 # triple pairs (kt slots 0..7)
NSLOT = NPAIR + 2 * NQ          # kt slots: 8 triple-pairs + 2 per quad

FUSED = (2, 5, 8, 16, 17, 18, 19)   # DVE fp32 fused chain (all quads anchored)
ACTS = tuple(i for i in range(NTILE) if i not in FUSED)  # 13 Act drains
# pack consecutive Act tiles into halves of wide bf16 buffers
APAIR = tuple((ACTS[2 * k], ACTS[2 * k + 1]) for k in range(len(ACTS) // 2))
ALONE = ACTS[-1]                # leftover narrow tile (15)
# production order: quads interleaved mid-btile so their serial DVE G-ops
# land in DVE's slack window; btile tail is all fast-release Act drains
PORDER = (0, 1, 2, 3, 16, 4, 5, 17, 6, 7, 18, 8, 9, 19, 10, 11, 12, 13, 14, 15)

_cached = {}


def _build_nc():
    import concourse.bass as bass
    import concourse.mybir as mybir
    import concourse.tile as tile
    from concourse import bacc

    fp32 = mybir.dt.float32
    fp16 = mybir.dt.float16
    bf16 = mybir.dt.bfloat16
    nc = bacc.Bacc("TRN2", target_bir_lowering=False, debug=False)

    xt_d = nc.dram_tensor("xt", [NBT, 128, NSLOT * 128], fp16, kind="ExternalInput").ap()
    kt_d = nc.dram_tensor("kt", [NSLOT, 128, RUL], fp16, kind="ExternalInput").ap()
    out_d = nc.dram_tensor("out", [BLOC, ULOC], fp32, kind="ExternalOutput").ap()

    W2 = 2 * RUL

    # map act tile -> (pair index, half) or None for the leftover
    half_of = {}
    for k, (i, j) in enumerate(APAIR):
        half_of[i] = (k, 0)
        half_of[j] = (k, 1)

    with tile.TileContext(nc) as tc:
        with (
            tc.tile_pool(name="kt", bufs=1) as ktpool,
            tc.tile_pool(name="xt", bufs=1) as xtpool,
            tc.tile_pool(name="fb", bufs=12) as fbpool,
            tc.tile_pool(name="fn", bufs=2) as fnpool,
            tc.tile_pool(name="gg", bufs=2) as ggpool,
            tc.tile_pool(name="tt", bufs=2) as ttpool,
            tc.tile_pool(name="uu", bufs=2) as uupool,
            tc.tile_pool(name="vv", bufs=2) as vvpool,
            tc.tile_pool(name="oo", bufs=2) as oopool,
            tc.tile_pool(name="out", bufs=2) as outpool,
            tc.tile_pool(name="ps", bufs=4, space="PSUM") as pspool,
        ):
            xts = [
                xtpool.tile([128, NSLOT * 128], fp16, tag=f"xt{t}", name=f"xt{t}")
                for t in range(NBT)
            ]
            kts = [
                ktpool.tile([128, RUL], fp16, tag=f"kt{s}", name=f"kt{s}")
                for s in range(NSLOT)
            ]
            # three DGE rings: sync + scalar (HW) and gpsimd (SW).  First
            # matmul needs xt0 cols 0:128 + kt0; split xt0 into consumption-
            # ordered column chunks and fan the urgent transfers out.
            # kt slots ordered by first use under PORDER; three DGE rings
            nc.scalar.dma_start(kts[0][:], kt_d[0])
            nc.sync.dma_start(xts[0][:], xt_d[0])
            for s in (1, 9, 2, 11, 3, 13, 4, 15):
                nc.gpsimd.dma_start(kts[s][:], kt_d[s])
            for s in (8, 10, 12, 14, 5, 6, 7):
                nc.sync.dma_start(kts[s][:], kt_d[s])
            for t in range(1, NBT):
                nc.sync.dma_start(xts[t][:], xt_d[t])

            pending = []

            def xsl(s):
                return slice(s * 128, (s + 1) * 128)

            for t in range(NBT):
                xt = xts[t]

                Gb = [
                    ggpool.tile([128, RUL], fp32, tag=f"G{i}", name=f"G{i}")
                    for i in range(2)
                ]
                Tb = [
                    ttpool.tile([128, W2], bf16, tag=f"T{i}", name=f"T{i}")
                    for i in range(2)
                ]
                Ub = [
                    uupool.tile([128, W2], bf16, tag=f"U{i}", name=f"U{i}")
                    for i in range(2)
                ]
                Vw = vvpool.tile([128, W2], bf16, tag="V", name="V")
                Sn = vvpool.tile([128, RUL], bf16, tag="Sn", name="Sn")
                fbs = [
                    fbpool.tile([128, W2], bf16, tag="fb", name="fb")
                    for _ in range(len(APAIR))
                ]
                fnn = fnpool.tile([128, RUL], bf16, tag="fn", name="fn")

                ng = 0
                for i in PORDER:
                    ps = pspool.tile([128, RUL], fp32, tag="ps", name="ps")
                    if i < NT:  # triple half: pair p = i//2, sub s = i%2
                        p, s = divmod(i, 2)
                        rows = slice(64 * s, 64 * s + 64)
                        for h in range(2):
                            cs = slice(512 * h, 512 * h + 512)
                            nc.tensor.matmul(
                                ps[:, cs],
                                xt[rows, xsl(p)],
                                kts[p][rows, cs],
                                start=True,
                                stop=True,
                                tile_position=(64 * s, 0),
                            )
                    else:  # quad: 2 psum-accumulated K=128 passes
                        q = i - NT
                        for h in range(2):
                            slot = NPAIR + 2 * q + h
                            for c in range(2):
                                cs = slice(512 * c, 512 * c + 512)
                                nc.tensor.matmul(
                                    ps[:, cs],
                                    xt[:, xsl(slot)],
                                    kts[slot][:, cs],
                                    start=(h == 0),
                                    stop=(h == 1),
                                )

                    if i in FUSED:
                        if ng == 0:
                            nc.vector.tensor_copy(Gb[0][:], ps[:])
                        else:
                            nc.vector.tensor_mul(
                                Gb[ng % 2][:], Gb[(ng - 1) % 2][:], ps[:]
                            )
                        ng += 1
                    elif i == ALONE:
                        nc.scalar.copy(fnn[:], ps[:])
                    else:
                        k, h = half_of[i]
                        nc.scalar.copy(fbs[k][:, h * RUL : (h + 1) * RUL], ps[:])
                        # Pool (slow, 4.1us/wide) gets the EARLY buffers so it
                        # finishes within the btile; DVE (fast) gets the LATE
                        # ones so the cross-btile tail is short.
                        if i == APAIR[1][1]:       # B0,B1 drained
                            nc.gpsimd.tensor_mul(Ub[0][:], fbs[0][:], fbs[1][:])
                        elif i == APAIR[2][1]:     # B2 drained
                            nc.gpsimd.tensor_mul(Ub[1][:], Ub[0][:], fbs[2][:])
                        elif i == APAIR[4][1]:     # B3,B4 drained
                            nc.vector.tensor_mul(Tb[0][:], fbs[3][:], fbs[4][:])
                        elif i == APAIR[5][1]:     # B5 drained
                            nc.vector.tensor_mul(Tb[1][:], Tb[0][:], fbs[5][:])

                    if i == 2 and pending:
                        pending.pop(0)()        # prev btile: Pool O, O2
                    if i == 8 and pending:
                        pending.pop(0)()        # prev btile: DVE reduce, dma

                    # tail right after T1 (Sn2 follows at the ALONE drain,
                    # which is the last position)
                    if i == APAIR[5][1]:
                        nc.vector.tensor_mul(Vw[:], Tb[1][:], Ub[1][:])
                        nc.vector.tensor_mul(Sn[:], Vw[:, 0:RUL], Vw[:, RUL:W2])

                O = oopool.tile([128, RUL], fp32, tag="O", name="O")
                O2 = oopool.tile([128, RUL], fp32, tag="O2", name="O2")

                def fin_pool(G=Gb[(len(FUSED) - 1) % 2], Sn=Sn, fnn=fnn, O=O, O2=O2):
                    nc.gpsimd.tensor_mul(O[:], G[:], Sn[:])
                    nc.gpsimd.tensor_mul(O2[:], O[:], fnn[:])

                def fin_reduce(t=t, O2=O2):
                    osum = outpool.tile([128, ULOC], fp32, tag="osum", name="osum")
                    nc.vector.tensor_reduce(
                        osum[:],
                        O2[:].rearrange("p (u r) -> p u r", r=R),
                        axis=mybir.AxisListType.X,
                        op=mybir.AluOpType.add,
                    )
                    nc.sync.dma_start(out_d[t * 128 : (t + 1) * 128, :], osum[:])

                pending.append(fin_pool)
                pending.append(fin_reduce)

            for fin in pending:
                fin()

    nc.compile()
    return nc


def _host_prep(X, K):
    """Repack inputs into per-core fp16 stationary/moving operands.

    Quad q covers features 4q..4q+3 as two K=128 PSUM-accumulated passes
    (row = ((d0*4+d1)*4+d2)*2 + l, l indexing half of the 4th feature's
    d range).  Triples cover features 48+3j..50+3j (row = d0*16+d1*4+d2),
    two per kt slot (rows 0:64 / 64:128) for row-tiled matmul pairs;
    feature 63 rides in the third pair's B half (rows 64:68).  Columns are
    u-major (col = u*32 + r).
    """
    f16 = np.float16
    FT = 4 * NQ                      # first triple feature
    kt_cores, xt_cores = [], []
    for bi in range(BSH):
        Xc = X[bi * BLOC : (bi + 1) * BLOC]                    # [512, 64, 4]
        for uj in range(USH):
            Ku = K[:, :, :, uj * ULOC : (uj + 1) * ULOC]       # [4,32,64,32]
            Kf = np.ascontiguousarray(
                Ku.transpose(2, 0, 3, 1).reshape(F, D, RUL)
            )                                                   # [f, d, col]
            kt = np.zeros((NSLOT, 128, RUL), dtype=f16)
            xt = np.zeros((NBT, 128, NSLOT * 128), dtype=f16)

            def put_x(slot, rows, arr):  # arr [BLOC, nrows]
                for t in range(NBT):
                    xt[t, rows, slot * 128 : (slot + 1) * 128] = arr[
                        t * 128 : (t + 1) * 128
                    ].T

            # triple pairs in slots 0..NPAIR-1
            for p in range(NPAIR):
                for s in range(2):
                    j = 2 * p + s
                    rows = slice(64 * s, 64 * s + 64)
                    f0 = FT + 3 * j
                    K3 = (
                        Kf[f0][:, None, None, :]
                        * Kf[f0 + 1][None, :, None, :]
                        * Kf[f0 + 2][None, None, :, :]
                    ).reshape(64, RUL)
                    X3 = (
                        Xc[:, f0, :, None, None]
                        * Xc[:, f0 + 1, None, :, None]
                        * Xc[:, f0 + 2, None, None, :]
                    ).reshape(BLOC, 64)
                    kt[p, rows] = K3
                    put_x(p, rows, X3)
            # quads in slots NPAIR + 2q + h
            for q in range(NQ):
                f0 = 4 * q
                K012 = (
                    Kf[f0][:, None, None, :]
                    * Kf[f0 + 1][None, :, None, :]
                    * Kf[f0 + 2][None, None, :, :]
                ).reshape(64, RUL)
                X012 = (
                    Xc[:, f0, :, None, None]
                    * Xc[:, f0 + 1, None, :, None]
                    * Xc[:, f0 + 2, None, None, :]
                ).reshape(BLOC, 64)
                for h in range(2):
                    slot = NPAIR + 2 * q + h
                    kt[slot] = (
                        K012[:, None, :] * Kf[f0 + 3][2 * h : 2 * h + 2][None, :, :]
                    ).reshape(128, RUL)
                    X4h = (
                        X012[:, :, None]
                        * Xc[:, f0 + 3, 2 * h : 2 * h + 2][:, None, :]
                    ).reshape(BLOC, 128)
                    put_x(slot, slice(0, 128), X4h)
            kt_cores.append(np.ascontiguousarray(kt))
            xt_cores.append(np.ascontiguousarray(xt))
    return [{"xt": xt_cores[c], "kt": kt_cores[c]} for c in range(NCORES)]


def kernel(**inputs):
    from concourse.bass_utils import run_bass_kernel_spmd

    X = np.asarray(inputs["X"], dtype=np.float32)
    K = np.asarray(inputs["kernel"], dtype=np.float32)
    assert X.shape == (B, F, D) and K.shape == (D, R, F, U)

    if "nc" not in _cached:
        _cached["nc"] = _build_nc()
    nc = _cached["nc"]

    in_maps = _host_prep(X, K)
    res = run_bass_kernel_spmd(nc, in_maps, core_ids=list(range(NCORES)))
    out = np.zeros((B, U), dtype=np.float32)
    for c in range(NCORES):
        bi, uj = divmod(c, USH)
        out[bi * BLOC : (bi + 1) * BLOC, uj * ULOC : (uj + 1) * ULOC] = res.results[
            c
        ]["out"]
    return out


# revision 33
# speedup vs baseline: 1.0357x; 1.0221x over previous
"""Trainium2 Bass kernel for nn_CP_Based (CP-decomposition interaction layer).

Math (full problem):
    t[b,f,r,u] = sum_d X[b,f,d] * K[d,r,f,u]      (B=1024, F=64, D=4, R=32, U=128)
    had[b,r,u] = prod_f t[b,f,r,u]
    out[b,u]   = sum_r had[b,r,u]

Strategy (v11):
  * Shard batch x units across 8 cores as (2 batch halves) x (4 unit
    quarters): per core B_loc=512 (4 partition tiles of 128) and RU_loc =
    32r x 32u = 1024 columns (u-major, r contiguous for the final reduce).
  * Host-side feature grouping: 16 triples (K=64, row-group pairs sharing a
    kt slot) + 4 quads (K=256 as 2 PSUM-accumulated K=128 passes) = 20
    factor tiles [128,1024] per batch tile.  Matmul inputs fp16.
  * PSUM ring: narrow [128,1024] tiles, bufs=4 (8 banks); all consumers are
    narrow single-tile ops so every bank releases independently.  The PE is
    HAM-throttled (1.2GHz cold / 2.4GHz warm); keeping the release path fast
    keeps the PE dense and warm.
  * Consumers: 7 tiles (incl. ALL 4 quads) fold into a narrow fp32 fused
    chain G on DVE; 13 tiles drain via Act into bf16, packed pairwise into
    [128,2048] buffers.  Quads are interleaved mid-btile (PORDER) so their
    serial G-ops sit in DVE's slack window and the btile tail is all
    fast-release Act drains.
  * Product tree is FOLD-FIRST and all-narrow: each wide buffer folds into
    a [128,1024] bf16 partial early (DVE 2x mode ~684ns; Pool ~2.1us),
    then narrow chain muls.  Pool handles the first 3 buffers (its ops
    spread mid-btile instead of clustering at the end); DVE the last 3 +
    the merge V.  Finalize (O = G*V, O2 = O*fnn on Pool; strided r-reduce
    on DVE) is deferred into the next btile in two stages.
  * Inputs are relaid out in CONSUMPTION order (PERM) so kt arrives as 3
    chunked contiguous DMAs and xt0 as 2 chunks across three DGE rings
    (sync + scalar HW rings, gpsimd SW ring): first matmul fires as soon as
    the first ~0.6MB lands.
"""

import numpy as np

B, F, D, R, U = 1024, 64, 4, 32, 128
NCORES = 8
BSH, USH = 2, 4                 # batch shards x unit shards
BLOC = B // BSH                 # 512 batch rows per core
NBT = BLOC // 128               # 4 batch tiles of 128
ULOC = U // USH                 # 32 units per core
RUL = R * ULOC                  # 1024 columns (u-major: col = u*32 + r)
NQ = 4                          # quads (features 0..15)
NT = 16                         # triples (features 16..63)
NTILE = NT + NQ                 # 20 factor tiles per batch tile
NPAIR = NT // 2                 # triple pairs
NSLOT = NPAIR + 2 * NQ          # kt slots: 8 triple-pairs + 2 per quad

FUSED = (2, 5, 8, 16, 17, 18, 19)   # DVE fp32 fused chain (all quads anchored)
ACTS = tuple(i for i in range(NTILE) if i not in FUSED)  # 13 Act drains
APAIR = tuple((ACTS[2 * k], ACTS[2 * k + 1]) for k in range(len(ACTS) // 2))
ALONE = ACTS[-1]                # leftover narrow tile (15)
# production order: quads interleaved mid-btile
PORDER = (0, 1, 2, 3, 16, 4, 5, 17, 6, 7, 18, 8, 9, 19, 10, 11, 12, 13, 14, 15)
# kt slot consumption order under PORDER (quad q uses slots 8+2q, 9+2q)
SLOT_ORDER = (0, 1, 8, 9, 2, 10, 11, 3, 12, 13, 4, 14, 15, 5, 6, 7)
SPOS = {s: p for p, s in enumerate(SLOT_ORDER)}   # orig slot -> position
KCHUNK = (4, 6, 6)              # kt DMA chunks (positions 0:4, 4:10, 10:16)
XSPLIT = 4                      # xt0 split after 4 slot-positions

_cached = {}


def _build_nc():
    import concourse.bass as bass
    import concourse.mybir as mybir
    import concourse.tile as tile
    from concourse import bacc

    fp32 = mybir.dt.float32
    fp16 = mybir.dt.float16
    bf16 = mybir.dt.bfloat16
    nc = bacc.Bacc("TRN2", target_bir_lowering=False, debug=False)

    # DRAM inputs are stored in consumption (PERM) order by _host_prep;
    # kt is partition-major so chunk DMAs get large contiguous descriptors
    xt_d = nc.dram_tensor("xt", [NBT, 128, NSLOT * 128], fp16, kind="ExternalInput").ap()
    kt_d = nc.dram_tensor("kt", [128, NSLOT * RUL], fp16, kind="ExternalInput").ap()
    out_d = nc.dram_tensor("out", [BLOC, ULOC], fp32, kind="ExternalOutput").ap()

    W2 = 2 * RUL

    half_of = {}
    for k, (i, j) in enumerate(APAIR):
        half_of[i] = (k, 0)
        half_of[j] = (k, 1)
    # fold trigger: second tile of each pair; buffer k consumer engine
    FOLD_AT = {APAIR[k][1]: k for k in range(len(APAIR))}

    with tile.TileContext(nc) as tc:
        with (
            tc.tile_pool(name="kt", bufs=1) as ktpool,
            tc.tile_pool(name="xt", bufs=1) as xtpool,
            tc.tile_pool(name="fb", bufs=12) as fbpool,
            tc.tile_pool(name="fn", bufs=2) as fnpool,
            tc.tile_pool(name="gg", bufs=2) as ggpool,
            tc.tile_pool(name="pf", bufs=2) as pfpool,
            tc.tile_pool(name="df", bufs=2) as dfpool,
            tc.tile_pool(name="cc", bufs=2) as ccpool,
            tc.tile_pool(name="dd", bufs=2) as ddpool,
            tc.tile_pool(name="vv", bufs=2) as vvpool,
            tc.tile_pool(name="oo", bufs=2) as oopool,
            tc.tile_pool(name="out", bufs=2) as outpool,
            tc.tile_pool(name="ps", bufs=4, space="PSUM") as pspool,
        ):
            # kt as 3 chunk tiles in PERM position order
            ktc = []
            base = 0
            for ci, n in enumerate(KCHUNK):
                ktc.append(
                    (base, ktpool.tile([128, n * RUL], fp16, tag=f"ktc{ci}",
                                       name=f"ktc{ci}"))
                )
                base += n

            def ktv(slot):
                p = SPOS[slot]
                for base, tile_ in reversed(ktc):
                    if p >= base:
                        off = (p - base) * RUL
                        return tile_[:, off : off + RUL]

            # xt0 split into 2 chunks by PERM position; xt1-3 whole
            xt0a = xtpool.tile([128, XSPLIT * 128], fp16, tag="xt0a", name="xt0a")
            xt0b = xtpool.tile([128, (NSLOT - XSPLIT) * 128], fp16, tag="xt0b",
                               name="xt0b")
            xts = [None] + [
                xtpool.tile([128, NSLOT * 128], fp16, tag=f"xt{t}", name=f"xt{t}")
                for t in range(1, NBT)
            ]

            def xtv(t, slot):
                p = SPOS[slot]
                if t == 0:
                    if p < XSPLIT:
                        return xt0a[:, p * 128 : (p + 1) * 128]
                    q = p - XSPLIT
                    return xt0b[:, q * 128 : (q + 1) * 128]
                return xts[t][:, p * 128 : (p + 1) * 128]

            # DMA: urgent first chunks fan across three rings
            c0, c1 = KCHUNK[0] * RUL, (KCHUNK[0] + KCHUNK[1]) * RUL
            nc.scalar.dma_start(xt0a[:], xt_d[0][:, 0 : XSPLIT * 128])
            nc.sync.dma_start(ktc[0][1][:], kt_d[:, 0:c0])
            nc.gpsimd.dma_start(ktc[1][1][:], kt_d[:, c0:c1])
            nc.sync.dma_start(xt0b[:], xt_d[0][:, XSPLIT * 128 :])
            nc.sync.dma_start(ktc[2][1][:], kt_d[:, c1:])
            for t in range(1, NBT):
                nc.sync.dma_start(xts[t][:], xt_d[t])

            pending = []

            for t in range(NBT):
                Gb = [
                    ggpool.tile([128, RUL], fp32, tag=f"G{i}", name=f"G{i}")
                    for i in range(2)
                ]
                PF = [
                    pfpool.tile([128, RUL], bf16, tag=f"PF{i}", name=f"PF{i}")
                    for i in range(3)
                ]
                DF = [
                    dfpool.tile([128, RUL], bf16, tag=f"DF{i}", name=f"DF{i}")
                    for i in range(3)
                ]
                Cb = [
                    ccpool.tile([128, RUL], bf16, tag=f"C{i}", name=f"C{i}")
                    for i in range(2)
                ]
                Db = [
                    ddpool.tile([128, RUL], bf16, tag=f"D{i}", name=f"D{i}")
                    for i in range(2)
                ]
                Vn = vvpool.tile([128, RUL], bf16, tag="V", name="V")
                fbs = [
                    fbpool.tile([128, W2], bf16, tag="fb", name="fb")
                    for _ in range(len(APAIR))
                ]
                fnn = fnpool.tile([128, RUL], bf16, tag="fn", name="fn")

                ng = 0
                for i in PORDER:
                    ps = pspool.tile([128, RUL], fp32, tag="ps", name="ps")
                    if i < NT:  # triple half: pair p = i//2, sub s = i%2
                        p, s = divmod(i, 2)
                        rows = slice(64 * s, 64 * s + 64)
                        for h in range(2):
                            cs = slice(512 * h, 512 * h + 512)
                            nc.tensor.matmul(
                                ps[:, cs],
                                xtv(t, p)[rows, :],
                                ktv(p)[rows, cs],
                                start=True,
                                stop=True,
                                tile_position=(64 * s, 0),
                            )
                    else:  # quad: 2 psum-accumulated K=128 passes
                        q = i - NT
                        for h in range(2):
                            slot = NPAIR + 2 * q + h
                            for c in range(2):
                                cs = slice(512 * c, 512 * c + 512)
                                nc.tensor.matmul(
                                    ps[:, cs],
                                    xtv(t, slot),
                                    ktv(slot)[:, cs],
                                    start=(h == 0),
                                    stop=(h == 1),
                                )

                    if i in FUSED:
                        if ng == 0:
                            nc.vector.tensor_copy(Gb[0][:], ps[:])
                        else:
                            nc.vector.tensor_mul(
                                Gb[ng % 2][:], Gb[(ng - 1) % 2][:], ps[:]
                            )
                        ng += 1
                    elif i == ALONE:
                        nc.scalar.copy(fnn[:], ps[:])
                    else:
                        k, h = half_of[i]
                        nc.scalar.copy(fbs[k][:, h * RUL : (h + 1) * RUL], ps[:])
                        if i in FOLD_AT:
                            k = FOLD_AT[i]
                            fb = fbs[k]
                            if k < 3:   # Pool side: fold + chain, spread early
                                nc.gpsimd.tensor_mul(
                                    PF[k][:], fb[:, 0:RUL], fb[:, RUL:W2]
                                )
                                if k == 1:
                                    nc.gpsimd.tensor_mul(
                                        Cb[0][:], PF[0][:], PF[1][:]
                                    )
                                elif k == 2:
                                    nc.gpsimd.tensor_mul(
                                        Cb[1][:], Cb[0][:], PF[2][:]
                                    )
                            else:       # DVE side: cheap narrow 2x folds
                                j = k - 3
                                nc.vector.tensor_mul(
                                    DF[j][:], fb[:, 0:RUL], fb[:, RUL:W2]
                                )
                                if j == 1:
                                    nc.vector.tensor_mul(
                                        Db[0][:], DF[0][:], DF[1][:]
                                    )
                                elif j == 2:
                                    nc.vector.tensor_mul(
                                        Db[1][:], Db[0][:], DF[2][:]
                                    )
                                    nc.vector.tensor_mul(
                                        Vn[:], Db[1][:], Cb[1][:]
                                    )

                    if i == 2 and pending:
                        pending.pop(0)()        # prev btile: Pool O, O2
                    if i == 8 and pending:
                        pending.pop(0)()        # prev btile: DVE reduce, dma

                O = oopool.tile([128, RUL], fp32, tag="O", name="O")
                O2 = oopool.tile([128, RUL], fp32, tag="O2", name="O2")

                def fin_pool(G=Gb[(len(FUSED) - 1) % 2], Vn=Vn, fnn=fnn, O=O, O2=O2):
                    nc.gpsimd.tensor_mul(O[:], G[:], Vn[:])
                    nc.gpsimd.tensor_mul(O2[:], O[:], fnn[:])

                def fin_reduce(t=t, O2=O2):
                    osum = outpool.tile([128, ULOC], fp32, tag="osum", name="osum")
                    nc.vector.tensor_reduce(
                        osum[:],
                        O2[:].rearrange("p (u r) -> p u r", r=R),
                        axis=mybir.AxisListType.X,
                        op=mybir.AluOpType.add,
                    )
                    nc.sync.dma_start(out_d[t * 128 : (t + 1) * 128, :], osum[:])

                pending.append(fin_pool)
                pending.append(fin_reduce)

            for fin in pending:
                fin()

    nc.compile()
    return nc


def _host_prep(X, K):
    """Repack inputs into per-core fp16 operands, in CONSUMPTION order.

    Factor construction is unchanged from earlier versions (triple slot p
    holds two row-group triples; quad q is two K=128 PSUM-accumulated
    passes), but both kt slots and xt column blocks are stored at PERM
    position SPOS[slot] so the device can fetch contiguous chunks in the
    order it consumes them.
    """
    f16 = np.float16
    FT = 4 * NQ                      # first triple feature
    kt_cores, xt_cores = [], []
    for bi in range(BSH):
        Xc = X[bi * BLOC : (bi + 1) * BLOC]                    # [512, 64, 4]
        for uj in range(USH):
            Ku = K[:, :, :, uj * ULOC : (uj + 1) * ULOC]       # [4,32,64,32]
            Kf = np.ascontiguousarray(
                Ku.transpose(2, 0, 3, 1).reshape(F, D, RUL)
            )                                                   # [f, d, col]
            kt = np.zeros((128, NSLOT * RUL), dtype=f16)
            xt = np.zeros((NBT, 128, NSLOT * 128), dtype=f16)

            def put_k(slot, rows, arr):  # arr [nrows, RUL]
                p = SPOS[slot]
                kt[rows, p * RUL : (p + 1) * RUL] = arr

            def put_x(slot, rows, arr):  # arr [BLOC, nrows]
                p = SPOS[slot]
                for t in range(NBT):
                    xt[t, rows, p * 128 : (p + 1) * 128] = arr[
                        t * 128 : (t + 1) * 128
                    ].T

            # triple pairs in slots 0..NPAIR-1
            for p in range(NPAIR):
                for s in range(2):
                    j = 2 * p + s
                    rows = slice(64 * s, 64 * s + 64)
                    f0 = FT + 3 * j
                    K3 = (
                        Kf[f0][:, None, None, :]
                        * Kf[f0 + 1][None, :, None, :]
                        * Kf[f0 + 2][None, None, :, :]
                    ).reshape(64, RUL)
                    X3 = (
                        Xc[:, f0, :, None, None]
                        * Xc[:, f0 + 1, None, :, None]
                        * Xc[:, f0 + 2, None, None, :]
                    ).reshape(BLOC, 64)
                    put_k(p, rows, K3)
                    put_x(p, rows, X3)
            # quads in slots NPAIR + 2q + h
            for q in range(NQ):
                f0 = 4 * q
                K012 = (
                    Kf[f0][:, None, None, :]
                    * Kf[f0 + 1][None, :, None, :]
                    * Kf[f0 + 2][None, None, :, :]
                ).reshape(64, RUL)
                X012 = (
                    Xc[:, f0, :, None, None]
                    * Xc[:, f0 + 1, None, :, None]
                    * Xc[:, f0 + 2, None, None, :]
                ).reshape(BLOC, 64)
                for h in range(2):
                    slot = NPAIR + 2 * q + h
                    put_k(slot, slice(0, 128), (
                        K012[:, None, :] * Kf[f0 + 3][2 * h : 2 * h + 2][None, :, :]
                    ).reshape(128, RUL))
                    X4h = (
                        X012[:, :, None]
                        * Xc[:, f0 + 3, 2 * h : 2 * h + 2][:, None, :]
                    ).reshape(BLOC, 128)
                    put_x(slot, slice(0, 128), X4h)
            kt_cores.append(np.ascontiguousarray(kt))
            xt_cores.append(np.ascontiguousarray(xt))
    return [{"xt": xt_cores[c], "kt": kt_cores[c]} for c in range(NCORES)]


def kernel(**inputs):
    from concourse.bass_utils import run_bass_kernel_spmd

    X = np.asarray(inputs["X"], dtype=np.float32)
    K = np.asarray(inputs["kernel"], dtype=np.float32)
    assert X.shape == (B, F, D) and K.shape == (D, R, F, U)

    if "nc" not in _cached:
        _cached["nc"] = _build_nc()
    nc = _cached["nc"]

    in_maps = _host_prep(X, K)
    res = run_bass_kernel_spmd(nc, in_maps, core_ids=list(range(NCORES)))
    out = np.zeros((B, U), dtype=np.float32)
    for c in range(NCORES):
        bi, uj = divmod(c, USH)
        out[bi * BLOC : (bi + 1) * BLOC, uj * ULOC : (uj + 1) * ULOC] = res.results[
            c
        ]["out"]
    return out
